# revision 22
# baseline (speedup 1.0000x reference)
"""BinaryConnect dense layer on 8 Trainium2 NeuronCores.

Computes Y = X @ sign(W) + bias for X[8192,4096], W[4096,4096] f32.

Strategy (data-parallel over X rows, 1024 rows/core):
- sign(W) in {-1,+1} is exact in any float dtype, so the matmul can run
  at the PE's fast-dtype rate. Device-side binarize: wb = (w >= 0) - 0.5
  in {-0.5,+0.5} (one DVE op); the final eviction scales by 2 (exact).
- mode "fp32r": one matmul pass with both operands viewed as float32r
  (full-rate fp32 replication mode on the PE).
- mode "hilo": X split on-device into hi = bf16(x), lo = bf16(x - hi);
  two bf16 passes accumulate hi@Wb + lo@Wb in fp32 PSUM (fp32-grade
  precision, 2x the PE work of fp32r).
- Each core computes Y_shard^T with binarized-W tiles stationary and the
  SBUF-resident X^T streamed repeatedly. W streams from HBM exactly once
  per core.
- Host transposes shards / concatenates results (layout only).
"""

import numpy as np

import concourse.bass as bass
import concourse.mybir as mybir
from concourse import bacc
from concourse.tile import TileContext
from concourse.bass_utils import run_bass_kernel_spmd

P = 128
N_CORES = 8
N_FULL = 8192
K_DIM = 4096
M_DIM = 4096
MODE = "fp16"


def build_bc_program(n_rows, k_dim, m_dim, mb_cols=256, n_free=512, mode="hilo",
                     kb=4, wb_bufs=4):
    """One-core SPMD program: yt[m_dim, n_rows] = (xt.T @ sign(w)).T + b.

    xt: [k_dim, n_rows] f32 (X shard, pre-transposed on host)
    w:  [k_dim, m_dim]  f32 (full weight)
    b:  [m_dim, 1]      f32
    kb: k-tiles batched per input DMA (fewer, larger transfers)
    """
    f32 = mybir.dt.float32
    f32r = mybir.dt.float32r
    bf16 = mybir.dt.bfloat16
    fp16 = mybir.dt.float16
    if mode == "fp32r":
        mb_cols = min(mb_cols, P)  # f32r wb blocks are 2x bf16 size
    if mode == "hilo":
        wb_bufs = 2                # SBUF is tight with resident hi+lo
        kb = min(kb, 2)
    if mode == "fp16":
        # 4-way m-tile interleave: 8 PSUM banks in flight keeps the PE
        # saturated while the X stream is still arriving.
        mb_cols = 512
        wb_bufs = 2
    KT = k_dim // P
    kb = min(kb, KT)
    KB_N = KT // kb               # input DMA batches
    MTPB = mb_cols // P        # m-tiles per binarize block
    NMB = m_dim // mb_cols     # number of binarize blocks
    nchunks = (n_rows + n_free - 1) // n_free
    chunk_sz = [min(n_free, n_rows - j * n_free) for j in range(nchunks)]

    nc = bacc.Bacc()
    xt = nc.declare_dram_parameter("xt", [k_dim, n_rows], f32, isOutput=False)
    w = nc.declare_dram_parameter("w", [k_dim, m_dim], f32, isOutput=False)
    b = nc.declare_dram_parameter("b", [m_dim, 1], f32, isOutput=False)
    yt = nc.declare_dram_parameter("yt", [m_dim, n_rows], f32, isOutput=True)

    wb_dt = {"hilo": bf16, "fp32r": f32r, "fp16": fp16}[mode]
    # DRAM views with the k-tile axis split out: [P, KT, cols]
    xt_r = xt.ap().rearrange("(kt p) n -> p kt n", p=P)
    w_r = w.ap().rearrange("(kt p) m -> p kt m", p=P)
    b_r = b.ap().rearrange("(mt p) o -> p mt o", p=P)

    with TileContext(nc) as tc:
        with (
            tc.tile_pool(name="xstage", bufs=2) as xstage_pool,
            tc.tile_pool(name="xres", bufs=1) as xres_pool,
            tc.tile_pool(name="wstage", bufs=2 if mode == "fp16" else 3) as wstage_pool,
            tc.tile_pool(name="wbp", bufs=wb_bufs) as wb_pool,
            tc.tile_pool(name="biasp", bufs=1) as bias_pool,
            tc.tile_pool(name="outp", bufs=3) as out_pool,
            tc.tile_pool(
                name="psump", bufs=max(1, 8 // (MTPB * nchunks)), space="PSUM"
            ) as psum_pool,
        ):
            bts = bias_pool.tile([P, m_dim // P, 1], f32, name="bts", tag="bts")
            nc.sync.dma_start(out=bts[:], in_=b_r[:, :, :])

            def produce_wb(mb):
                # Binarize W block mb: wb = (w >= 0) - 0.5 in {-0.5, +0.5}.
                wb = wb_pool.tile([P, KT * mb_cols], wb_dt, name="wb", tag="wb")
                for kg in range(KB_N):
                    wf = wstage_pool.tile([P, kb, mb_cols], f32, name="wf", tag="wf")
                    nc.sync.dma_start(
                        out=wf[:],
                        in_=w_r[:, kg * kb:(kg + 1) * kb,
                                mb * mb_cols:(mb + 1) * mb_cols],
                    )
                    for t in range(kb):
                        k = kg * kb + t
                        nc.vector.tensor_scalar(
                            out=wb[:, k * mb_cols:(k + 1) * mb_cols],
                            in0=wf[:, t, :],
                            scalar1=0.0,
                            scalar2=0.5,
                            op0=mybir.AluOpType.is_ge,
                            op1=mybir.AluOpType.subtract,
                        )
                return wb

            # First weight block before the X stream so the PE can start
            # as soon as the first X k-slices land.
            wb_tiles = {0: produce_wb(0)}

            if mode == "hilo":
                # Phase 0: load X^T, split into resident hi/lo bf16.
                xhi = xres_pool.tile([P, KT * n_rows], bf16, name="xhi", tag="xhi")
                xlo = xres_pool.tile([P, KT * n_rows], bf16, name="xlo", tag="xlo")
                for kg in range(KB_N):
                    xf = xstage_pool.tile([P, kb, n_rows], f32, name="xf", tag="xf")
                    nc.sync.dma_start(
                        out=xf[:], in_=xt_r[:, kg * kb:(kg + 1) * kb, :]
                    )
                    for t in range(kb):
                        k = kg * kb + t
                        hi = xhi[:, k * n_rows:(k + 1) * n_rows]
                        lo = xlo[:, k * n_rows:(k + 1) * n_rows]
                        nc.vector.tensor_copy(out=hi, in_=xf[:, t, :])
                        nc.vector.tensor_sub(out=lo, in0=xf[:, t, :], in1=hi)
            else:
                # Phase 0: load X^T, round to the resident matmul dtype
                # (float32r: 11-bit mantissa; float16: 10-bit) via DVE copy.
                x_dt = f32r if mode == "fp32r" else fp16
                xall = xres_pool.tile([P, KT * n_rows], x_dt, name="xall", tag="xall")
                for kg in range(KB_N):
                    xf = xstage_pool.tile([P, kb, n_rows], f32, name="xf", tag="xf")
                    nc.sync.dma_start(
                        out=xf[:], in_=xt_r[:, kg * kb:(kg + 1) * kb, :]
                    )
                    for t in range(kb):
                        k = kg * kb + t
                        nc.vector.tensor_copy(
                            out=xall[:, k * n_rows:(k + 1) * n_rows],
                            in_=xf[:, t, :],
                        )

            for mb in range(NMB):
                wb = wb_tiles.pop(mb)
                if mb + 1 < NMB:
                    # Prefetch the next weight block ahead of this block's
                    # matmuls so its DMAs overlap the PE work.
                    wb_tiles[mb + 1] = produce_wb(mb + 1)

                # All MTPB m-tiles of this block accumulate together,
                # k-interleaved, so the PE has MTPB*nchunks matmuls per
                # arriving X k-slice during the phase-0 DMA stream.
                psums = [
                    [
                        psum_pool.tile(
                            [P, chunk_sz[j]], f32,
                            name=f"ps{mi}_{j}", tag=f"ps{mi}_{j}",
                        )
                        for j in range(nchunks)
                    ]
                    for mi in range(MTPB)
                ]
                for k in range(KT):
                    for mi in range(MTPB):
                        lhsT = wb[:, k * mb_cols + mi * P:k * mb_cols + (mi + 1) * P]
                        if mode == "hilo":
                            for j in range(nchunks):
                                c0 = k * n_rows + j * n_free
                                rh = xhi[:, c0:c0 + chunk_sz[j]]
                                rl = xlo[:, c0:c0 + chunk_sz[j]]
                                nc.tensor.matmul(
                                    psums[mi][j][:], lhsT, rh,
                                    start=(k == 0), stop=False,
                                )
                                nc.tensor.matmul(
                                    psums[mi][j][:], lhsT, rl,
                                    start=False, stop=(k == KT - 1),
                                )
                        else:
                            for j in range(nchunks):
                                c0 = k * n_rows + j * n_free
                                rr = xall[:, c0:c0 + chunk_sz[j]]
                                nc.tensor.matmul(
                                    psums[mi][j][:], lhsT, rr,
                                    start=(k == 0), stop=(k == KT - 1),
                                )
                # Evict: y = 2 * psum + bias (exact), then DMA out.
                for mi in range(MTPB):
                    m = mb * MTPB + mi
                    out_t = out_pool.tile([P, n_rows], f32, name="out_t", tag="out_t")
                    for j in range(nchunks):
                        nc.vector.tensor_scalar(
                            out=out_t[:, j * n_free:j * n_free + chunk_sz[j]],
                            in0=psums[mi][j][:],
                            scalar1=2.0,
                            scalar2=bts[:, m, :],
                            op0=mybir.AluOpType.mult,
                            op1=mybir.AluOpType.add,
                        )
                    nc.sync.dma_start(out=yt[m * P:(m + 1) * P, :], in_=out_t[:])
    nc.compile()
    return nc


_NC_CACHE = {}


def _get_program(mode=None):
    if mode is None:
        mode = MODE
    key = (N_FULL // N_CORES, K_DIM, M_DIM, mode)
    if key not in _NC_CACHE:
        _NC_CACHE[key] = build_bc_program(*key[:3], mode=mode)
    return _NC_CACHE[key]


def make_in_maps(x, w, b):
    rows = x.shape[0] // N_CORES
    w = np.ascontiguousarray(np.asarray(w, dtype=np.float32))
    b = np.ascontiguousarray(np.asarray(b, dtype=np.float32).reshape(-1, 1))
    in_maps = []
    for c in range(N_CORES):
        shard = np.ascontiguousarray(
            np.asarray(x[c * rows:(c + 1) * rows, :], dtype=np.float32).T
        )
        in_maps.append({"xt": shard, "w": w, "b": b})
    return in_maps


def assemble_output(results, n_full=N_FULL, m_dim=M_DIM):
    rows = n_full // N_CORES
    y = np.empty((n_full, m_dim), dtype=np.float32)
    for c in range(N_CORES):
        y[c * rows:(c + 1) * rows, :] = results[c]["yt"].T
    return y


def kernel(x, kernel, bias):
    nc = _get_program()
    in_maps = make_in_maps(x, kernel, bias)
    res = run_bass_kernel_spmd(nc, in_maps, list(range(N_CORES)))
    return assemble_output(res.results)


# revision 26
# speedup vs baseline: 1.0217x; 1.0217x over previous
"""BinaryConnect dense layer on 8 Trainium2 NeuronCores.

Computes Y = X @ sign(W) + bias for X[8192,4096], W[4096,4096] f32.

Strategy (data-parallel over X rows, 1024 rows/core):
- sign(W) in {-1,+1} is exact in any float dtype, so the matmul can run
  at the PE's fast-dtype rate. Device-side binarize: wb = (w >= 0) - 0.5
  in {-0.5,+0.5} (one DVE op); the final eviction scales by 2 (exact).
- mode "fp32r": one matmul pass with both operands viewed as float32r
  (full-rate fp32 replication mode on the PE).
- mode "hilo": X split on-device into hi = bf16(x), lo = bf16(x - hi);
  two bf16 passes accumulate hi@Wb + lo@Wb in fp32 PSUM (fp32-grade
  precision, 2x the PE work of fp32r).
- Each core computes Y_shard^T with binarized-W tiles stationary and the
  SBUF-resident X^T streamed repeatedly. W streams from HBM exactly once
  per core.
- Host transposes shards / concatenates results (layout only).
"""

import numpy as np

import concourse.bass as bass
import concourse.mybir as mybir
from concourse import bacc
from concourse.tile import TileContext
from concourse.bass_utils import run_bass_kernel_spmd

P = 128
N_CORES = 8
N_FULL = 8192
K_DIM = 4096
M_DIM = 4096
MODE = "fp16"


def build_bc_program(n_rows, k_dim, m_dim, mb_cols=256, n_free=512, mode="hilo",
                     kb=4, wb_bufs=4):
    """One-core SPMD program: yt[m_dim, n_rows] = (xt.T @ sign(w)).T + b.

    xt: [k_dim, n_rows] f32 (X shard, pre-transposed on host)
    w:  [k_dim, m_dim]  f32 (full weight)
    b:  [m_dim, 1]      f32
    kb: k-tiles batched per input DMA (fewer, larger transfers)
    """
    f32 = mybir.dt.float32
    f32r = mybir.dt.float32r
    bf16 = mybir.dt.bfloat16
    fp16 = mybir.dt.float16
    if mode == "fp32r":
        mb_cols = min(mb_cols, P)  # f32r wb blocks are 2x bf16 size
    if mode == "hilo":
        wb_bufs = 2                # SBUF is tight with resident hi+lo
        kb = min(kb, 2)

    KT = k_dim // P
    kb = min(kb, KT)
    KB_N = KT // kb               # input DMA batches
    MTPB = mb_cols // P        # m-tiles per binarize block
    NMB = m_dim // mb_cols     # number of binarize blocks
    nchunks = (n_rows + n_free - 1) // n_free
    chunk_sz = [min(n_free, n_rows - j * n_free) for j in range(nchunks)]

    nc = bacc.Bacc()
    xt = nc.declare_dram_parameter("xt", [k_dim, n_rows], f32, isOutput=False)
    w = nc.declare_dram_parameter("w", [k_dim, m_dim], f32, isOutput=False)
    b = nc.declare_dram_parameter("b", [m_dim, 1], f32, isOutput=False)
    yt = nc.declare_dram_parameter("yt", [m_dim, n_rows], f32, isOutput=True)

    wb_dt = {"hilo": bf16, "fp32r": f32r, "fp16": fp16}[mode]
    # DRAM views with the k-tile axis split out: [P, KT, cols]
    xt_r = xt.ap().rearrange("(kt p) n -> p kt n", p=P)
    w_r = w.ap().rearrange("(kt p) m -> p kt m", p=P)
    b_r = b.ap().rearrange("(mt p) o -> p mt o", p=P)

    with TileContext(nc) as tc:
        with (
            tc.tile_pool(name="xstage", bufs=2) as xstage_pool,
            tc.tile_pool(name="xres", bufs=1) as xres_pool,
            tc.tile_pool(name="wstage", bufs=2 if mode == "fp16" else 3) as wstage_pool,
            tc.tile_pool(name="wbp", bufs=wb_bufs) as wb_pool,
            tc.tile_pool(name="biasp", bufs=1) as bias_pool,
            tc.tile_pool(name="outp", bufs=3) as out_pool,
            tc.tile_pool(
                name="psump", bufs=max(1, 8 // (MTPB * nchunks)), space="PSUM"
            ) as psum_pool,
        ):
            bts = bias_pool.tile([P, m_dim // P, 1], f32, name="bts", tag="bts")
            nc.sync.dma_start(out=bts[:], in_=b_r[:, :, :])

            def produce_wb(mb):
                # Binarize W block mb: wb = (w >= 0) - 0.5 in {-0.5, +0.5}.
                wb = wb_pool.tile([P, KT * mb_cols], wb_dt, name="wb", tag="wb")
                for kg in range(KB_N):
                    wf = wstage_pool.tile([P, kb, mb_cols], f32, name="wf", tag="wf")
                    nc.sync.dma_start(
                        out=wf[:],
                        in_=w_r[:, kg * kb:(kg + 1) * kb,
                                mb * mb_cols:(mb + 1) * mb_cols],
                    )
                    for t in range(kb):
                        k = kg * kb + t
                        nc.vector.tensor_scalar(
                            out=wb[:, k * mb_cols:(k + 1) * mb_cols],
                            in0=wf[:, t, :],
                            scalar1=0.0,
                            scalar2=0.5,
                            op0=mybir.AluOpType.is_ge,
                            op1=mybir.AluOpType.subtract,
                        )
                return wb

            # First weight block before the X stream so the PE can start
            # as soon as the first X k-slices land.
            wb_tiles = {0: produce_wb(0)}

            if mode == "hilo":
                # Phase 0: load X^T, split into resident hi/lo bf16.
                xhi = xres_pool.tile([P, KT * n_rows], bf16, name="xhi", tag="xhi")
                xlo = xres_pool.tile([P, KT * n_rows], bf16, name="xlo", tag="xlo")
                for kg in range(KB_N):
                    xf = xstage_pool.tile([P, kb, n_rows], f32, name="xf", tag="xf")
                    nc.sync.dma_start(
                        out=xf[:], in_=xt_r[:, kg * kb:(kg + 1) * kb, :]
                    )
                    for t in range(kb):
                        k = kg * kb + t
                        hi = xhi[:, k * n_rows:(k + 1) * n_rows]
                        lo = xlo[:, k * n_rows:(k + 1) * n_rows]
                        nc.vector.tensor_copy(out=hi, in_=xf[:, t, :])
                        nc.vector.tensor_sub(out=lo, in0=xf[:, t, :], in1=hi)
            else:
                # Phase 0: load X^T, round to the resident matmul dtype
                # (float32r: 11-bit mantissa; float16: 10-bit) via DVE copy.
                x_dt = f32r if mode == "fp32r" else fp16
                xall = xres_pool.tile([P, KT * n_rows], x_dt, name="xall", tag="xall")
                for kg in range(KB_N):
                    xf = xstage_pool.tile([P, kb, n_rows], f32, name="xf", tag="xf")
                    nc.sync.dma_start(
                        out=xf[:], in_=xt_r[:, kg * kb:(kg + 1) * kb, :]
                    )
                    for t in range(kb):
                        k = kg * kb + t
                        # On ACT: keeps DVE free for weight binarization
                        # during the startup window.
                        nc.scalar.copy(
                            out=xall[:, k * n_rows:(k + 1) * n_rows],
                            in_=xf[:, t, :],
                        )

            for mb in range(NMB):
                wb = wb_tiles.pop(mb)
                if mb != 0 and mb + 1 < NMB:
                    # Prefetch the next weight block ahead of this block's
                    # matmuls so its DMAs overlap the PE work. Block 1 is
                    # deferred (emitted after block 0's matmuls) so the X
                    # stream owns HBM bandwidth during startup.
                    wb_tiles[mb + 1] = produce_wb(mb + 1)

                # All MTPB m-tiles of this block accumulate together,
                # k-interleaved, so the PE has MTPB*nchunks matmuls per
                # arriving X k-slice during the phase-0 DMA stream.
                psums = [
                    [
                        psum_pool.tile(
                            [P, chunk_sz[j]], f32,
                            name=f"ps{mi}_{j}", tag=f"ps{mi}_{j}",
                        )
                        for j in range(nchunks)
                    ]
                    for mi in range(MTPB)
                ]
                for k in range(KT):
                    for mi in range(MTPB):
                        lhsT = wb[:, k * mb_cols + mi * P:k * mb_cols + (mi + 1) * P]
                        if mode == "hilo":
                            for j in range(nchunks):
                                c0 = k * n_rows + j * n_free
                                rh = xhi[:, c0:c0 + chunk_sz[j]]
                                rl = xlo[:, c0:c0 + chunk_sz[j]]
                                nc.tensor.matmul(
                                    psums[mi][j][:], lhsT, rh,
                                    start=(k == 0), stop=False,
                                )
                                nc.tensor.matmul(
                                    psums[mi][j][:], lhsT, rl,
                                    start=False, stop=(k == KT - 1),
                                )
                        else:
                            for j in range(nchunks):
                                c0 = k * n_rows + j * n_free
                                rr = xall[:, c0:c0 + chunk_sz[j]]
                                nc.tensor.matmul(
                                    psums[mi][j][:], lhsT, rr,
                                    start=(k == 0), stop=(k == KT - 1),
                                )
                # Evict on ACT: y = Identity(2 * psum + bias) (exact), DMA out.
                for mi in range(MTPB):
                    m = mb * MTPB + mi
                    out_t = out_pool.tile([P, n_rows], f32, name="out_t", tag="out_t")
                    for j in range(nchunks):
                        nc.scalar.activation(
                            out=out_t[:, j * n_free:j * n_free + chunk_sz[j]],
                            in_=psums[mi][j][:],
                            func=mybir.ActivationFunctionType.Identity,
                            bias=bts[:, m, :],
                            scale=2.0,
                        )
                    nc.sync.dma_start(out=yt[m * P:(m + 1) * P, :], in_=out_t[:])
                if mb == 0 and mb + 1 < NMB:
                    wb_tiles[mb + 1] = produce_wb(mb + 1)
    nc.compile()
    return nc


_NC_CACHE = {}


def _get_program(mode=None):
    if mode is None:
        mode = MODE
    key = (N_FULL // N_CORES, K_DIM, M_DIM, mode)
    if key not in _NC_CACHE:
        _NC_CACHE[key] = build_bc_program(*key[:3], mode=mode)
    return _NC_CACHE[key]


def make_in_maps(x, w, b):
    rows = x.shape[0] // N_CORES
    w = np.ascontiguousarray(np.asarray(w, dtype=np.float32))
    b = np.ascontiguousarray(np.asarray(b, dtype=np.float32).reshape(-1, 1))
    in_maps = []
    for c in range(N_CORES):
        shard = np.ascontiguousarray(
            np.asarray(x[c * rows:(c + 1) * rows, :], dtype=np.float32).T
        )
        in_maps.append({"xt": shard, "w": w, "b": b})
    return in_maps


def assemble_output(results, n_full=N_FULL, m_dim=M_DIM):
    rows = n_full // N_CORES
    y = np.empty((n_full, m_dim), dtype=np.float32)
    for c in range(N_CORES):
        y[c * rows:(c + 1) * rows, :] = results[c]["yt"].T
    return y


def kernel(x, kernel, bias):
    nc = _get_program()
    in_maps = make_in_maps(x, kernel, bias)
    res = run_bass_kernel_spmd(nc, in_maps, list(range(N_CORES)))
    return assemble_output(res.results)
